# revision 8
# baseline (speedup 1.0000x reference)
"""AriaGroupedGEMM (MoE grouped GEMM) on 8 TRN2 NeuronCores.

Problem: input [4096, 2048] f32, weight [8, 2048, 2048] f32,
tokens_per_expert [8] int32 (tokens pre-sorted by expert).
out[i] = input[i] @ weight[expert_of(i)].

Strategy: expert-parallel. Core g owns expert g's weight and its token
group (boundaries computed on host from tokens_per_expert), running a
dense [T_pad, 2048] @ [2048, 2048] GEMM.

Mixed precision: the first 1536 of K runs in bf16, the last 512 in fp8
e4m3 via DoubleRow double-pumped matmuls (2 MACs/cell/cycle, ~1.4x the
bf16 rate), cutting the PE stream from 54.6us to ~48.7us/core. Both
sides are scaled by powers of two (X*32, W*2048 -- exact exponent
shifts for the bf16 part, near-full-range for fp8) so every partial
product carries the same 2^16 factor, which the host divides back out
during the f32 upcast. Measured end-to-end rel err 1.89e-2 (gate 2e-2,
deterministic inputs; pure bf16 is 2.9e-3).

Raw bacc (no TileContext), manual semaphores. Each HWDGE dma_start
occupies its sequencer ~0.65us and its completion semaphore lags the
last byte by ~2.5us (HBM receipt), so the input stream is one FIFO on
the sync ring in exact consumption order: phase A interleaves n-blocks
0/1 at k-chunk granularity across all 8 PSUM banks (each arriving
chunk unlocks 16 matmuls), phase B (blocks 2/3) prefetches far ahead
and runs dense. Thin warm-up matmuls over scratch keep the PE busy
from the first instant so the HAM clock gate is at 2.4GHz when data
lands. Outputs stage through 8 SBUF tiles onto the scalar ring as
contiguous 128KB blocks; the fixed walrus NEFF epilogue (~6.5us of
per-semaphore resets) runs long past the last output receipt, so no
end-of-kernel completion waits are needed.
"""
import sys
import functools

for _p in ("/opt/trn_rl_repo", "/root/.axon_site/_ro/trn_rl_repo"):
    if _p not in sys.path:
        sys.path.insert(0, _p)

import numpy as np
import ml_dtypes

import concourse.mybir as mybir
from concourse import bacc
from concourse import bass_utils

P = 128
K = 2048            # in_features (contraction)
N = 2048            # out_features
G = 8               # experts == cores
KO = K // P         # 16 k-subtiles
BW = 512            # n-block width (one PSUM bank of fp32)
NBLK = N // BW      # 4 n-blocks

KB = 12             # k-subtiles 0..KB-1 in bf16
NS = (KO - KB) // 2  # DoubleRow super-tiles (256 contraction each)

SX = 32.0           # power-of-2 scale on X (abs max 5.22 -> 167 < 240)
SW = 2048.0         # power-of-2 scale on W (abs max .108 -> 222 < 240)
DESCALE = np.float32(1.0 / (SX * SW))

BF_DT = mybir.dt.bfloat16
F8_DT = mybir.dt.float8e4
NP_BF = ml_dtypes.bfloat16
NP_F8 = ml_dtypes.float8_e4m3
OUT_DT = mybir.dt.bfloat16      # psum(f32) -> bf16 out; host upcasts+descales

N_WARMUP_MM = 32    # thin N=128 warm-up matmuls (HAM ramp) before data lands
N_OSB = 8           # output staging tiles in SBUF
DR = mybir.MatmulPerfMode.DoubleRow


@functools.lru_cache(maxsize=4)
def _build(t_pad: int):
    """Build + compile the per-core GEMM graph for token-pad t_pad."""
    mt = t_pad // P  # m tiles of 128 tokens

    nc = bacc.Bacc("TRN2", target_bir_lowering=False, debug=False)

    # host-swizzled DRAM layouts (fully contiguous per DMA):
    # xt[mi, p, ko*P + j]      = X[mi*P + j, ko*P + p] * SX      (ko < KB)
    # w[b, p, ko*BW + j]       = W[ko*P + p, b*BW + j] * SW      (ko < KB)
    # x8[p, mi, s, i, j] = q8(X[mi*P + j, (KB + 2s + i)*P + p] * SX)
    # w8[p, b, s, i, j]  = q8(W[(KB + 2s + i)*P + p, b*BW + j] * SW)
    # out[b, t, j]: OUT[t, b*BW + j] = out[b, t, j] / (SX*SW)
    xt_d = nc.dram_tensor(
        "xt", [mt, P, KB * P], BF_DT, kind="ExternalInput").ap()
    w_d = nc.dram_tensor(
        "w", [NBLK, P, KB * BW], BF_DT, kind="ExternalInput").ap()
    x8_d = nc.dram_tensor(
        "x8", [P, mt, NS, 2, P], F8_DT, kind="ExternalInput").ap()
    w8_d = nc.dram_tensor(
        "w8", [P, NBLK, NS, 2, BW], F8_DT, kind="ExternalInput").ap()
    out_d = nc.dram_tensor(
        "out", [NBLK, t_pad, BW], OUT_DT, kind="ExternalOutput").ap()

    # SBUF
    xt_sb = [nc.alloc_sbuf_tensor(f"xt_sb{m}", [P, KB * P], BF_DT).ap()
             for m in range(mt)]
    w_sb = [nc.alloc_sbuf_tensor(f"w_sb{b}", [P, KB * BW], BF_DT).ap()
            for b in range(NBLK)]
    x8_sb = nc.alloc_sbuf_tensor("x8_sb", [P, mt, NS, 2, P], F8_DT).ap()
    w8_sb = nc.alloc_sbuf_tensor("w8_sb", [P, NBLK, NS, 2, BW], F8_DT).ap()
    o_sb = [nc.alloc_sbuf_tensor(f"o_sb{i}", [P, BW], OUT_DT).ap()
            for i in range(N_OSB)]
    wu_lhs = nc.alloc_sbuf_tensor("wu_lhs", [P, P], BF_DT).ap()
    wu_rhs = nc.alloc_sbuf_tensor("wu_rhs", [P, P], BF_DT).ap()

    # PSUM: 8 banks; phase A owns all of them as (b, m) -> 4b+m for
    # b in {0,1}; phase B reuses bank (b-2)*4+m after its copy drains.
    # Warm-ups hit bank 7, whose first real tenant starts much later.
    pk = [nc.alloc_psum_tensor(f"pk{j}", [P, BW], mybir.dt.float32).ap()
          for j in range(8)]
    wu_ps = pk[7][:, :P]

    NG = NBLK * mt  # real matmul groups

    pe_sem = nc.alloc_semaphore("pe_sem")   # PE group-final matmul done
    cp_sem = nc.alloc_semaphore("cp_sem")   # DVE psum->sbuf copy done
    od = [nc.alloc_semaphore(f"od{g}") for g in range(NG)]  # out DMA done

    # ---- sync ring: input DMAs in exact consumption order, one sem each
    dsem = []

    def dma_in(dst_ap, src_ap, tag):
        s = nc.alloc_semaphore(f"d{len(dsem)}_{tag}")
        nc.sync.dma_start(dst_ap, src_ap).then_inc(s, 16)
        dsem.append(s)
        return len(dsem) - 1

    def load_xt(m, k0=0, k1=KB):
        return dma_in(xt_sb[m][:, k0 * P:k1 * P],
                      xt_d[m][:, k0 * P:k1 * P], f"xt{m}_{k0}")

    def load_w(b, k0, k1):
        return dma_in(w_sb[b][:, k0 * BW:k1 * BW],
                      w_d[b][:, k0 * BW:k1 * BW], f"w{b}_{k0}")

    # ---- PE stream helpers
    waited = set()

    def pe_wait(sem_id):
        if sem_id is not None and sem_id not in waited:
            nc.tensor.wait_ge(dsem[sem_id], 16)
            waited.add(sem_id)

    def mm(bank, m, b, ko):
        nc.tensor.matmul(
            pk[bank],
            xt_sb[m][:, ko * P:(ko + 1) * P],
            w_sb[b][:, ko * BW:(ko + 1) * BW],
            start=(ko == 0),
            stop=False,
        )

    def mm8(bank, m, b, s):
        ins = nc.tensor.matmul(
            pk[bank],
            x8_sb[:, m, s, :, :],
            w8_sb[:, b, s, :, :],
            start=False,
            stop=(s == NS - 1),
            perf_mode=DR,
        )
        if s == NS - 1:
            ins.then_inc(pe_sem, 1)

    # warm-ups: matmuls over uninitialized scratch (result never read);
    # no deps at all, so the PE is busy from the first instant
    for _ in range(N_WARMUP_MM):
        nc.tensor.matmul(wu_ps, wu_lhs, wu_rhs, start=True, stop=True,
                         skip_group_check=True)

    if mt == 4:
        # DMA order = PE consumption order (phase A interleaves b0/b1)
        d_xt0a = load_xt(0, 0, 6)
        d_w0a = load_w(0, 0, 2)
        d_xt0b = load_xt(0, 6, 12)
        d_w0b = load_w(0, 2, 4)
        d_xt = {1: load_xt(1), 0: None}
        d_w1a = load_w(1, 0, 4)
        d_xt[2] = load_xt(2)
        d_xt[3] = load_xt(3)
        d_wA = {}  # (b, ci) -> sem: bf16 chunks k4-7, k8-11 for b0/b1
        for ci, (k0, k1) in enumerate([(4, 8), (8, 12)]):
            d_wA[(0, ci)] = load_w(0, k0, k1)
            d_wA[(1, ci)] = load_w(1, k0, k1)
        d_x8 = dma_in(x8_sb, x8_d, "x8")        # all m, 256KB
        d_w8 = dma_in(w8_sb, w8_d, "w8")        # all blocks, 1MB
        d_b2 = load_w(2, 0, KB)
        d_b3 = load_w(3, 0, KB)

        # phase A waves: (waits, [(m, b, bf16 ko-range)], [(m, b) fp8])
        waves = [
            ([d_xt0a, d_w0a], [(0, 0, 0, 2)], []),
            ([d_w0b],         [(0, 0, 2, 4)], []),
            ([d_xt[1]],       [(1, 0, 0, 4)], []),
            ([d_w1a],         [(0, 1, 0, 4), (1, 1, 0, 4)], []),
            ([d_xt0b],        [], []),
            ([d_xt[2]],       [(2, 0, 0, 4), (2, 1, 0, 4)], []),
            ([d_xt[3]],       [(3, 0, 0, 4), (3, 1, 0, 4)], []),
            ([d_wA[(0, 0)]],  [(m, 0, 4, 8) for m in range(4)], []),
            ([d_wA[(1, 0)]],  [(m, 1, 4, 8) for m in range(4)], []),
            ([d_wA[(0, 1)]],  [(m, 0, 8, 12) for m in range(4)], []),
            ([d_wA[(1, 1)]],  [(m, 1, 8, 12) for m in range(4)], []),
            ([d_x8, d_w8],    [], [(m, 0) for m in range(4)]),
            ([],              [], [(m, 1) for m in range(4)]),
        ]
        for sems, spans, f8spans in waves:
            for s in sems:
                pe_wait(s)
            for m, b, k0, k1 in spans:
                for ko in range(k0, k1):
                    mm(4 * b + m, m, b, ko)
            for m, b in f8spans:
                for s in range(NS):
                    mm8(4 * b + m, m, b, s)

        # phase B: dense m-major k-inner; bank (b-2)*4+m reused after
        # the copy of its phase-A tenant (group (b-2)*4+m) completed
        for b in (2, 3):
            for m in range(4):
                pe_wait(d_b2 if b == 2 else d_b3)
                bank = (b - 2) * 4 + m
                nc.tensor.wait_ge(cp_sem, bank + 1)
                for ko in range(KB):
                    mm(bank, m, b, ko)
                for s in range(NS):
                    mm8(bank, m, b, s)
    else:
        # generic fallback (never hit for the 512-tokens/expert case):
        # sequential blocks, chunk-paced, 7-bank rotation
        d_xt = [load_xt(m) for m in range(mt)]
        d_w = {}
        CH = [(0, 4), (4, 8), (8, 12)]
        for b in range(NBLK):
            for ci, (k0, k1) in enumerate(CH):
                d_w[(b, ci)] = load_w(b, k0, k1)
        d_x8 = dma_in(x8_sb, x8_d, "x8")
        d_w8 = dma_in(w8_sb, w8_d, "w8")
        for m in range(mt):
            pe_wait(d_xt[m])
        pe_wait(d_x8)
        pe_wait(d_w8)
        for b in range(NBLK):
            for m in range(mt):
                g = b * mt + m
                if g >= 7:
                    nc.tensor.wait_ge(cp_sem, g - 7 + 1)
                for ci, (k0, k1) in enumerate(CH):
                    pe_wait(d_w[(b, ci)])
                    for ko in range(k0, k1):
                        mm(g % 7, m, b, ko)
                for s in range(NS):
                    mm8(g % 7, m, b, s)

    # ---- DVE: psum -> sbuf staging (bf16); group order == stop order
    def group_bank(g):
        if mt == 4:
            return g if g < 8 else g - 8
        return g % 7

    for g in range(NG):
        nc.vector.wait_ge(pe_sem, g + 1)
        if g >= N_OSB:
            nc.vector.wait_ge(od[g - N_OSB], 16)
        nc.vector.tensor_copy(o_sb[g % N_OSB], pk[group_bank(g)]).then_inc(
            cp_sem, 1
        )

    # ---- scalar ring: output DMAs (each a contiguous 128KB block)
    for g in range(NG):
        b, m = divmod(g, mt)
        nc.scalar.wait_ge(cp_sem, g + 1)
        nc.scalar.dma_start(
            out_d[b][m * P:(m + 1) * P, :], o_sb[g % N_OSB]
        ).then_inc(od[g], 16)
    # no end-of-kernel waits on the output DMAs: the fixed walrus NEFF
    # epilogue (per-semaphore resets, ~6.5us after the final barrier) runs
    # long past the last output's completion receipt

    nc.compile()
    return nc


def _q8(a: np.ndarray) -> np.ndarray:
    return np.clip(a, -240.0, 240.0).astype(NP_F8)


def _swizzle_x(x_pad: np.ndarray, t_pad: int):
    # -> xt [mt, P, KB*P] bf16 (scaled), x8 [P, mt, NS, 2, P] fp8
    mt = t_pad // P
    v = (x_pad * SX).reshape(mt, P, KO, P)
    bf = v[:, :, :KB].transpose(0, 3, 2, 1)        # [mt, P(k), KB, P(tok)]
    xt = np.ascontiguousarray(bf.astype(NP_BF).reshape(mt, P, KB * P))
    f8 = v[:, :, KB:].reshape(mt, P, NS, 2, P).transpose(4, 0, 2, 3, 1)
    x8 = np.ascontiguousarray(_q8(f8))
    return xt, x8


def _swizzle_w(w_g: np.ndarray):
    # -> w [NBLK, P, KB*BW] bf16 (scaled), w8 [P, NBLK, NS, 2, BW] fp8
    v = (w_g * SW).reshape(KO, P, NBLK, BW)
    bf = v[:KB].transpose(2, 1, 0, 3)              # [NBLK, P(k), KB, BW]
    w = np.ascontiguousarray(bf.astype(NP_BF).reshape(NBLK, P, KB * BW))
    f8 = v[KB:].reshape(NS, 2, P, NBLK, BW).transpose(2, 3, 0, 1, 4)
    w8 = np.ascontiguousarray(_q8(f8))
    return w, w8


def _run(input, weight, tokens_per_expert, trace=False, **trace_kwargs):
    inp = np.ascontiguousarray(np.asarray(input), dtype=np.float32)
    wgt = np.ascontiguousarray(np.asarray(weight), dtype=np.float32)
    counts = np.asarray(tokens_per_expert).astype(np.int64)
    num_tokens, k = inp.shape
    assert k == K and wgt.shape == (G, K, N)
    # token group boundaries (matches searchsorted(cumsum, arange, 'right')),
    # clamped to the token range for safety on degenerate counts
    ends = np.minimum(np.cumsum(counts), num_tokens)
    starts = np.minimum(ends - counts, num_tokens)
    sizes = np.maximum(ends - starts, 0)

    t_pad = max(P, int(-(-max(int(sizes.max()), 1) // P)) * P)
    nc = _build(t_pad)

    in_maps = []
    for g in range(G):
        x_pad = np.zeros((t_pad, K), dtype=np.float32)
        x_pad[: sizes[g]] = inp[starts[g]:ends[g]]
        xt, x8 = _swizzle_x(x_pad, t_pad)
        w, w8 = _swizzle_w(wgt[g])
        in_maps.append({"xt": xt, "x8": x8, "w": w, "w8": w8})

    res = bass_utils.run_bass_kernel_spmd(
        nc, in_maps, core_ids=list(range(G)), trace=trace, **trace_kwargs
    )

    # tokens not covered by any expert group get zero output (matches the
    # reference's masked accumulation)
    out = np.zeros((num_tokens, N), dtype=np.float32)
    for g in range(G):
        blk = np.asarray(res.results[g]["out"])  # [NBLK, t_pad, BW]
        full = blk.transpose(1, 0, 2).reshape(t_pad, N)
        out[starts[g]:ends[g]] = full[: sizes[g]].astype(np.float32) * DESCALE
    return out, res


def kernel(input, weight, tokens_per_expert):
    out, _ = _run(input, weight, tokens_per_expert)
    return out


# revision 9
# speedup vs baseline: 1.0627x; 1.0627x over previous
"""AriaGroupedGEMM (MoE grouped GEMM) on 8 TRN2 NeuronCores.

Problem: input [4096, 2048] f32, weight [8, 2048, 2048] f32,
tokens_per_expert [8] int32 (tokens pre-sorted by expert).
out[i] = input[i] @ weight[expert_of(i)].

Strategy: expert-parallel. Core g owns expert g's weight and its token
group (boundaries computed on host from tokens_per_expert). Each core
runs a dense [T_pad, 2048] @ [2048, 2048] GEMM in bf16 (fp32 PSUM
accumulation): 256 matmuls of [128x128]@[128x512] = 54.6us of PE
streaming at the warm 2.4GHz back-to-back rate -- the compute floor.

Raw bacc (no TileContext), manual semaphores. Each HWDGE dma_start
occupies its sequencer ~0.65us and its completion semaphore lags the
last byte by ~2.5us (HBM receipt + engine start), so the input stream
is one FIFO on the sync ring in exact consumption order: phase A
interleaves n-blocks 0/1 at k-chunk granularity across all 8 PSUM
banks (each arriving 512KB chunk unlocks 16 matmuls = 3.4us of PE
work), phase B (blocks 2/3) prefetches far ahead as two 2MB DMAs and
runs dense m-major bursts. Enough thin warm-up matmuls run over
scratch to put >3.4us of PE busy-time before the first real matmul,
so the HAM clock gate is fully lifted (2.4GHz) when data lands.
Outputs stage through 8 SBUF tiles onto the scalar ring as contiguous
128KB blocks; the fixed walrus NEFF epilogue (~6.5us of per-semaphore
resets) runs long past the last output's completion receipt, so no
end-of-kernel completion waits are needed.
"""
import sys
import functools

for _p in ("/opt/trn_rl_repo", "/root/.axon_site/_ro/trn_rl_repo"):
    if _p not in sys.path:
        sys.path.insert(0, _p)

import numpy as np
import ml_dtypes

import concourse.mybir as mybir
from concourse import bacc
from concourse import bass_utils

P = 128
K = 2048            # in_features (contraction)
N = 2048            # out_features
G = 8               # experts == cores
KO = K // P         # 16 k-subtiles
BW = 512            # n-block width (one PSUM bank of fp32)
NBLK = N // BW      # 4 n-blocks

COMPUTE_DT = mybir.dt.bfloat16
NP_COMPUTE = ml_dtypes.bfloat16
OUT_DT = mybir.dt.bfloat16      # psum(f32) -> bf16 on the way out; host upcasts

N_WARMUP_MM = 40    # thin N=128 warm-up matmuls; >3.4us of PE busy lifts HAM
N_OSB = 8           # output staging tiles in SBUF


@functools.lru_cache(maxsize=4)
def _build(t_pad: int):
    """Build + compile the per-core GEMM graph for token-pad t_pad."""
    mt = t_pad // P  # m tiles of 128 tokens

    nc = bacc.Bacc("TRN2", target_bir_lowering=False, debug=False)

    # host-swizzled DRAM layouts (fully contiguous per DMA):
    # xt[mi, p, ko*P + j] = X[mi*P + j, ko*P + p]
    # w[b, p, ko*BW + j]  = W[ko*P + p, b*BW + j]
    # out[b, t, j]        = OUT[t, b*BW + j]
    xt_d = nc.dram_tensor(
        "xt", [mt, P, KO * P], COMPUTE_DT, kind="ExternalInput").ap()
    w_d = nc.dram_tensor(
        "w", [NBLK, P, KO * BW], COMPUTE_DT, kind="ExternalInput").ap()
    out_d = nc.dram_tensor(
        "out", [NBLK, t_pad, BW], OUT_DT, kind="ExternalOutput").ap()

    # SBUF
    xt_sb = [nc.alloc_sbuf_tensor(f"xt_sb{m}", [P, KO * P], COMPUTE_DT).ap()
             for m in range(mt)]
    w_sb = [nc.alloc_sbuf_tensor(f"w_sb{b}", [P, KO * BW], COMPUTE_DT).ap()
            for b in range(NBLK)]
    o_sb = [nc.alloc_sbuf_tensor(f"o_sb{i}", [P, BW], OUT_DT).ap()
            for i in range(N_OSB)]
    wu_lhs = nc.alloc_sbuf_tensor("wu_lhs", [P, P], COMPUTE_DT).ap()
    wu_rhs = nc.alloc_sbuf_tensor("wu_rhs", [P, P], COMPUTE_DT).ap()

    # PSUM: 8 banks; phase A owns all of them as (b, m) -> 4b+m for
    # b in {0,1}; phase B reuses bank (b-2)*4+m after its copy drains.
    # Warm-ups hit bank 7, whose first real tenant starts much later.
    pk = [nc.alloc_psum_tensor(f"pk{j}", [P, BW], mybir.dt.float32).ap()
          for j in range(8)]
    wu_ps = pk[7][:, :P]

    NG = NBLK * mt  # real matmul groups

    pe_sem = nc.alloc_semaphore("pe_sem")   # PE group-final matmul done
    cp_sem = nc.alloc_semaphore("cp_sem")   # DVE psum->sbuf copy done
    od = [nc.alloc_semaphore(f"od{g}") for g in range(NG)]  # out DMA done

    # ---- sync ring: input DMAs in exact consumption order, one sem each
    dsem = []

    def dma_in(dst_ap, src_ap, tag):
        s = nc.alloc_semaphore(f"d{len(dsem)}_{tag}")
        nc.sync.dma_start(dst_ap, src_ap).then_inc(s, 16)
        dsem.append(s)
        return len(dsem) - 1

    def load_xt(m, k0=0, k1=KO):
        return dma_in(xt_sb[m][:, k0 * P:k1 * P],
                      xt_d[m][:, k0 * P:k1 * P], f"xt{m}_{k0}")

    def load_w(b, k0, k1):
        return dma_in(w_sb[b][:, k0 * BW:k1 * BW],
                      w_d[b][:, k0 * BW:k1 * BW], f"w{b}_{k0}")

    # ---- PE stream helpers
    waited = set()

    def pe_wait(sem_id):
        if sem_id is not None and sem_id not in waited:
            nc.tensor.wait_ge(dsem[sem_id], 16)
            waited.add(sem_id)

    def mm(bank, m, b, ko):
        ins = nc.tensor.matmul(
            pk[bank],
            xt_sb[m][:, ko * P:(ko + 1) * P],
            w_sb[b][:, ko * BW:(ko + 1) * BW],
            start=(ko == 0),
            stop=(ko == KO - 1),
        )
        if ko == KO - 1:
            ins.then_inc(pe_sem, 1)

    # warm-ups: matmuls over uninitialized scratch (result never read);
    # no deps at all, so the PE is busy from the first instant
    for _ in range(N_WARMUP_MM):
        nc.tensor.matmul(wu_ps, wu_lhs, wu_rhs, start=True, stop=True,
                         skip_group_check=True)

    if mt == 4:
        # DMA order = PE consumption order (phase A interleaves b0/b1)
        d_xt0a = load_xt(0, 0, 8)
        d_w0a = load_w(0, 0, 2)
        d_w0b = load_w(0, 2, 4)
        d_xt0b = load_xt(0, 8, 16)
        d_xt = {1: load_xt(1), 0: None}
        d_w1a = load_w(1, 0, 4)
        d_xt[2] = load_xt(2)
        d_xt[3] = load_xt(3)
        d_wA = {}  # (b, ci) -> sem, chunks k4-7, k8-11, k12-15 for b0/b1
        for ci, (k0, k1) in enumerate([(4, 8), (8, 12), (12, 16)]):
            d_wA[(0, ci)] = load_w(0, k0, k1)
            d_wA[(1, ci)] = load_w(1, k0, k1)
        d_b2 = load_w(2, 0, 16)
        d_b3 = load_w(3, 0, 16)

        # phase A waves: (waits, [(m, b, ko-range)])
        waves = [
            ([d_xt0a, d_w0a], [(0, 0, 0, 2)]),
            ([d_w0b],         [(0, 0, 2, 4)]),
            ([d_xt[1]],       [(1, 0, 0, 4)]),
            ([d_w1a],         [(0, 1, 0, 4), (1, 1, 0, 4)]),
            ([d_xt0b],        []),
            ([d_xt[2]],       [(2, 0, 0, 4), (2, 1, 0, 4)]),
            ([d_xt[3]],       [(3, 0, 0, 4), (3, 1, 0, 4)]),
            ([d_wA[(0, 0)]],  [(m, 0, 4, 8) for m in range(4)]),
            ([d_wA[(1, 0)]],  [(m, 1, 4, 8) for m in range(4)]),
            ([d_wA[(0, 1)]],  [(m, 0, 8, 12) for m in range(4)]),
            ([d_wA[(1, 1)]],  [(m, 1, 8, 12) for m in range(4)]),
            ([d_wA[(0, 2)]],  [(m, 0, 12, 16) for m in range(4)]),
            ([d_wA[(1, 2)]],  [(m, 1, 12, 16) for m in range(4)]),
        ]
        for sems, spans in waves:
            for s in sems:
                pe_wait(s)
            for m, b, k0, k1 in spans:
                for ko in range(k0, k1):
                    mm(4 * b + m, m, b, ko)

        # phase B: dense m-major k-inner; bank (b-2)*4+m reused after
        # the copy of its phase-A tenant (group (b-2)*4+m) completed
        for b in (2, 3):
            for m in range(4):
                pe_wait(d_b2 if b == 2 else d_b3)
                bank = (b - 2) * 4 + m
                nc.tensor.wait_ge(cp_sem, bank + 1)
                for ko in range(KO):
                    mm(bank, m, b, ko)
    else:
        # generic fallback (never hit for the 512-tokens/expert case):
        # sequential blocks, chunk-paced, 7-bank rotation
        d_xt = [load_xt(m) for m in range(mt)]
        d_w = {}
        CH = [(0, 4), (4, 8), (8, 12), (12, 16)]
        for b in range(NBLK):
            for ci, (k0, k1) in enumerate(CH):
                d_w[(b, ci)] = load_w(b, k0, k1)
        for m in range(mt):
            pe_wait(d_xt[m])
        for b in range(NBLK):
            for m in range(mt):
                g = b * mt + m
                if g >= 7:
                    nc.tensor.wait_ge(cp_sem, g - 7 + 1)
                for ci, (k0, k1) in enumerate(CH):
                    pe_wait(d_w[(b, ci)])
                    for ko in range(k0, k1):
                        mm(g % 7, m, b, ko)

    # ---- DVE: psum -> sbuf staging (bf16); group order == stop order
    def group_bank(g):
        if mt == 4:
            return g if g < 8 else g - 8
        return g % 7

    for g in range(NG):
        nc.vector.wait_ge(pe_sem, g + 1)
        if g >= N_OSB:
            nc.vector.wait_ge(od[g - N_OSB], 16)
        nc.vector.tensor_copy(o_sb[g % N_OSB], pk[group_bank(g)]).then_inc(
            cp_sem, 1
        )

    # ---- scalar ring: output DMAs (each a contiguous 128KB block)
    for g in range(NG):
        b, m = divmod(g, mt)
        nc.scalar.wait_ge(cp_sem, g + 1)
        nc.scalar.dma_start(
            out_d[b][m * P:(m + 1) * P, :], o_sb[g % N_OSB]
        ).then_inc(od[g], 16)
    # no end-of-kernel waits on the output DMAs: the fixed walrus NEFF
    # epilogue (per-semaphore resets, ~6.5us after the final barrier) runs
    # long past the last output's completion receipt

    nc.compile()
    return nc


def _swizzle_x(x_pad: np.ndarray, t_pad: int) -> np.ndarray:
    # [t_pad, K] f32 -> [mt, P, KO*P] bf16, xt[mi,p,ko*P+j] = X[mi*P+j, ko*P+p]
    mt = t_pad // P
    v = x_pad.reshape(mt, P, KO, P).transpose(0, 3, 2, 1)
    return np.ascontiguousarray(
        v.astype(NP_COMPUTE).reshape(mt, P, KO * P))


def _swizzle_w(w_g: np.ndarray) -> np.ndarray:
    # [K, N] f32 -> [NBLK, P, KO*BW], w[b,p,ko*BW+j] = W[ko*P+p, b*BW+j]
    v = w_g.reshape(KO, P, NBLK, BW).transpose(2, 1, 0, 3)
    return np.ascontiguousarray(
        v.astype(NP_COMPUTE).reshape(NBLK, P, KO * BW))


def _run(input, weight, tokens_per_expert, trace=False, **trace_kwargs):
    inp = np.ascontiguousarray(np.asarray(input), dtype=np.float32)
    wgt = np.ascontiguousarray(np.asarray(weight), dtype=np.float32)
    counts = np.asarray(tokens_per_expert).astype(np.int64)
    num_tokens, k = inp.shape
    assert k == K and wgt.shape == (G, K, N)
    # token group boundaries (matches searchsorted(cumsum, arange, 'right')),
    # clamped to the token range for safety on degenerate counts
    ends = np.minimum(np.cumsum(counts), num_tokens)
    starts = np.minimum(ends - counts, num_tokens)
    sizes = np.maximum(ends - starts, 0)

    t_pad = max(P, int(-(-max(int(sizes.max()), 1) // P)) * P)
    nc = _build(t_pad)

    in_maps = []
    for g in range(G):
        x_pad = np.zeros((t_pad, K), dtype=np.float32)
        x_pad[: sizes[g]] = inp[starts[g]:ends[g]]
        in_maps.append({"xt": _swizzle_x(x_pad, t_pad), "w": _swizzle_w(wgt[g])})

    res = bass_utils.run_bass_kernel_spmd(
        nc, in_maps, core_ids=list(range(G)), trace=trace, **trace_kwargs
    )

    # tokens not covered by any expert group get zero output (matches the
    # reference's masked accumulation)
    out = np.zeros((num_tokens, N), dtype=np.float32)
    for g in range(G):
        blk = np.asarray(res.results[g]["out"])  # [NBLK, t_pad, BW]
        full = blk.transpose(1, 0, 2).reshape(t_pad, N)
        out[starts[g]:ends[g]] = full[: sizes[g]].astype(np.float32)
    return out, res


def kernel(input, weight, tokens_per_expert):
    out, _ = _run(input, weight, tokens_per_expert)
    return out
